# revision 3
# baseline (speedup 1.0000x reference)
"""AttentionGate (conv1x1 + InstanceNorm + ELU gate) on 8 trn2 NeuronCores.

Strategy:
  - Shard the D axis (32) across 8 cores (4 planes each); batch n=2 handled
    sequentially per core.
  - Host converts g/x and weights to bf16 (halves HBM traffic; all matmuls
    run bf16 with f32 PSUM accumulation).
  - InstanceNorm bias cancels, so bg/bx are mathematically irrelevant.
  - Per n: Phase 1 computes y_g = Wg.T@g (stored bf16 in SBUF) and
    y_x = Wx.T@x (stats only), accumulating per-chunk bn_stats. A tiny
    AllReduce (8 cores) merges [mean, E[y^2]] per (n, channel). Phase 2
    recomputes y_x from the SBUF-stored bf16 x, applies
    elu(z) = max(z,0) + min(exp(z),1) - 1 with per-channel constants folded
    into the psi bias, computes psi = sigmoid(Wpsi.T@(g1+x1)+bpsi) via
    tanh (same ACT table as Exp), broadcasts psi over partitions, and
    writes out = x * psi.
"""

import sys

if "/opt/trn_rl_repo/concourse" not in sys.path:
    sys.path.insert(0, "/opt/trn_rl_repo/concourse")

import contextlib

import numpy as np
import ml_dtypes

import concourse.bass as bass
import concourse.bacc as bacc
import concourse.mybir as mybir
import concourse.tile as tile
from concourse.bass_utils import run_bass_kernel_spmd

F32 = mybir.dt.float32
BF16 = mybir.dt.bfloat16
BF = ml_dtypes.bfloat16
AF = mybir.ActivationFunctionType
OP = mybir.AluOpType

N_CORES = 8
NB = 2          # batch
C = 320         # input channels
O = 160         # inter channels
EPS = 1e-5
CH = 512        # pixels per chunk


def build_kernel(S, n_cores=N_CORES):
    """Build the per-core bass kernel. S = per-core pixels per batch entry."""
    NCH = S // CH          # chunks per n
    NSC = NCH // 4         # superchunks (4 chunks) per n
    assert S % (4 * CH) == 0
    S_GLOBAL = S * n_cores

    nc = bacc.Bacc("TRN2", target_bir_lowering=False, debug=False,
                   num_devices=n_cores)

    g_d = nc.dram_tensor("g", [NB, C, S], BF16, kind="ExternalInput")
    x_d = nc.dram_tensor("x", [NB, C, S], BF16, kind="ExternalInput")
    wg_d = nc.dram_tensor("wgt", [C, O], BF16, kind="ExternalInput")
    wx_d = nc.dram_tensor("wxt", [C, O], BF16, kind="ExternalInput")
    wp_d = nc.dram_tensor("wpt", [O, 1], BF16, kind="ExternalInput")
    cb_d = nc.dram_tensor("cb", [1, 1], F32, kind="ExternalInput")
    out_d = nc.dram_tensor("out", [NB, C, S], F32, kind="ExternalOutput")

    ar_in = [nc.dram_tensor(f"ar_in{n}", [128, 8], F32, kind="Internal")
             for n in range(NB)]
    ar_out = [nc.dram_tensor(f"ar_out{n}", [128, 8], F32, kind="Internal",
                             addr_space="Shared")
              for n in range(NB)]

    with tile.TileContext(nc) as tc, contextlib.ExitStack() as ctx:
        cpool = ctx.enter_context(tc.tile_pool(name="cpool", bufs=1))
        store = ctx.enter_context(tc.tile_pool(name="store", bufs=1))
        stats = ctx.enter_context(tc.tile_pool(name="stats", bufs=1))
        inp = ctx.enter_context(tc.tile_pool(name="inp", bufs=3))
        xc2p_pool = ctx.enter_context(tc.tile_pool(name="xc2p", bufs=8))
        stage = ctx.enter_context(tc.tile_pool(name="stage", bufs=1))
        psip = ctx.enter_context(tc.tile_pool(name="psip", bufs=2))
        outp = ctx.enter_context(tc.tile_pool(name="outp", bufs=2))
        tiny = ctx.enter_context(tc.tile_pool(name="tiny", bufs=1))
        ps = ctx.enter_context(tc.tile_pool(name="ps", bufs=2, space="PSUM"))

        # ---- constants / weights ----
        wg0 = cpool.tile([128, O], BF16, tag="wg0")
        wg1 = cpool.tile([128, O], BF16, tag="wg1")
        wg2 = cpool.tile([64, O], BF16, tag="wg2")
        wx0 = cpool.tile([128, O], BF16, tag="wx0")
        wx1 = cpool.tile([128, O], BF16, tag="wx1")
        wx2 = cpool.tile([64, O], BF16, tag="wx2")
        nc.sync.dma_start(wg0[:], wg_d[0:128, :])
        nc.sync.dma_start(wg1[:], wg_d[128:256, :])
        nc.sync.dma_start(wg2[:], wg_d[256:320, :])
        nc.sync.dma_start(wx0[:], wx_d[0:128, :])
        nc.sync.dma_start(wx1[:], wx_d[128:256, :])
        nc.sync.dma_start(wx2[:], wx_d[256:320, :])
        wp0 = cpool.tile([128, 1], BF16, tag="wp0")
        wp1 = cpool.tile([32, 1], BF16, tag="wp1")
        nc.sync.dma_start(wp0[:], wp_d[0:128, :])
        nc.sync.dma_start(wp1[:], wp_d[128:160, :])
        cbh = cpool.tile([1, 1], F32, tag="cbh")
        nc.sync.dma_start(cbh[:], cb_d[:])

        # ---- persistent per-n storage (reused; deps serialize) ----
        xs0 = store.tile([128, S], BF16, tag="xs0")
        xs1 = store.tile([128, S], BF16, tag="xs1")
        yg0 = store.tile([128, S], BF16, tag="yg0")
        yg1 = store.tile([32, S], BF16, tag="yg1")

        # stats collect tiles
        sg0 = stats.tile([128, NCH * 6], F32, tag="sg0")
        sg1 = stats.tile([32, NCH * 6], F32, tag="sg1")
        sx0 = stats.tile([128, NCH * 6], F32, tag="sx0")
        sx1 = stats.tile([32, NCH * 6], F32, tag="sx1")

        for n in range(NB):
            # ================= Phase 1 =================
            for j in range(NCH):
                w0, w1 = j * CH, (j + 1) * CH
                ga = inp.tile([128, CH], BF16, tag="ga")
                gb = inp.tile([128, CH], BF16, tag="gb")
                gc = inp.tile([64, CH], BF16, tag="gc")
                nc.sync.dma_start(ga[:], g_d[n, 0:128, w0:w1])
                nc.sync.dma_start(gb[:], g_d[n, 128:256, w0:w1])
                nc.sync.dma_start(gc[:], g_d[n, 256:320, w0:w1])
                nc.sync.dma_start(xs0[:, w0:w1], x_d[n, 0:128, w0:w1])
                nc.sync.dma_start(xs1[:, w0:w1], x_d[n, 128:256, w0:w1])
                xc2 = xc2p_pool.tile([64, CH], BF16, tag="xc2")
                nc.sync.dma_start(xc2[:], x_d[n, 256:320, w0:w1])

                pg0 = ps.tile([128, CH], F32, tag="pA")
                nc.tensor.matmul(pg0[:], wg0[:, 0:128], ga[:], start=True, stop=False)
                nc.tensor.matmul(pg0[:], wg1[:, 0:128], gb[:], start=False, stop=False)
                nc.tensor.matmul(pg0[:], wg2[:, 0:128], gc[:], start=False, stop=True)
                pg1 = ps.tile([32, CH], F32, tag="pB")
                nc.tensor.matmul(pg1[:], wg0[:, 128:160], ga[:], start=True, stop=False)
                nc.tensor.matmul(pg1[:], wg1[:, 128:160], gb[:], start=False, stop=False)
                nc.tensor.matmul(pg1[:], wg2[:, 128:160], gc[:], start=False, stop=True)
                px0 = ps.tile([128, CH], F32, tag="pC")
                nc.tensor.matmul(px0[:], wx0[:, 0:128], xs0[:, w0:w1], start=True, stop=False)
                nc.tensor.matmul(px0[:], wx1[:, 0:128], xs1[:, w0:w1], start=False, stop=False)
                nc.tensor.matmul(px0[:], wx2[:, 0:128], xc2[:], start=False, stop=True)
                px1 = ps.tile([32, CH], F32, tag="pD")
                nc.tensor.matmul(px1[:], wx0[:, 128:160], xs0[:, w0:w1], start=True, stop=False)
                nc.tensor.matmul(px1[:], wx1[:, 128:160], xs1[:, w0:w1], start=False, stop=False)
                nc.tensor.matmul(px1[:], wx2[:, 128:160], xc2[:], start=False, stop=True)

                nc.vector.bn_stats(sg0[:, j * 6:(j + 1) * 6], pg0[:])
                nc.vector.bn_stats(sg1[:, j * 6:(j + 1) * 6], pg1[:])
                nc.vector.bn_stats(sx0[:, j * 6:(j + 1) * 6], px0[:])
                nc.vector.bn_stats(sx1[:, j * 6:(j + 1) * 6], px1[:])

                nc.scalar.activation(yg0[:, w0:w1], pg0[:], AF.Copy)
                nc.scalar.activation(yg1[:, w0:w1], pg1[:], AF.Copy)

            # ================= stats + AllReduce =================
            mv_g0 = tiny.tile([128, 2], F32, tag="mv_g0")
            mv_g1 = tiny.tile([32, 2], F32, tag="mv_g1")
            mv_x0 = tiny.tile([128, 2], F32, tag="mv_x0")
            mv_x1 = tiny.tile([32, 2], F32, tag="mv_x1")
            nc.vector.bn_aggr(mv_g0[:], sg0[:].rearrange("p (n s) -> p n s", s=6))
            nc.vector.bn_aggr(mv_g1[:], sg1[:].rearrange("p (n s) -> p n s", s=6))
            nc.vector.bn_aggr(mv_x0[:], sx0[:].rearrange("p (n s) -> p n s", s=6))
            nc.vector.bn_aggr(mv_x1[:], sx1[:].rearrange("p (n s) -> p n s", s=6))

            arst = tiny.tile([128, 8], F32, tag="arst")
            nc.vector.memset(arst[:], 0.0)
            for (mv, mcol, ecol, p) in (
                (mv_g0, 0, 1, 128), (mv_x0, 2, 3, 128),
                (mv_g1, 4, 5, 32), (mv_x1, 6, 7, 32),
            ):
                nc.vector.tensor_copy(arst[0:p, mcol:mcol + 1], mv[:, 0:1])
                # E[y^2] = var + mean^2
                nc.vector.tensor_tensor(arst[0:p, ecol:ecol + 1], mv[:, 0:1],
                                        mv[:, 0:1], OP.mult)
                nc.vector.tensor_tensor(arst[0:p, ecol:ecol + 1],
                                        arst[0:p, ecol:ecol + 1],
                                        mv[:, 1:2], OP.add)

            nc.sync.dma_start(ar_in[n].ap(), arst[:])
            nc.gpsimd.collective_compute(
                "AllReduce", OP.add,
                replica_groups=[list(range(n_cores))],
                ins=[ar_in[n].ap().opt()],
                outs=[ar_out[n].ap().opt()],
            )
            arb = tiny.tile([128, 8], F32, tag="arb")
            nc.sync.dma_start(arb[:], ar_out[n].ap())

            # ---- per-group norm constants ----
            inv = 1.0 / n_cores
            grp = {}
            for (name, mcol, ecol, p) in (
                ("g0", 0, 1, 128), ("x0", 2, 3, 128),
                ("g1", 4, 5, 32), ("x1", 6, 7, 32),
            ):
                mu = tiny.tile([p, 1], F32, tag=f"mu_{name}")
                nc.vector.tensor_scalar(mu[:], arb[0:p, mcol:mcol + 1],
                                        inv, None, OP.mult)
                var = tiny.tile([p, 1], F32, tag=f"var_{name}")
                # var = E2/ncores - mu^2 + EPS
                nc.vector.tensor_tensor(var[:], mu[:], mu[:], OP.mult)
                nc.vector.tensor_scalar(var[:], var[:], -1.0, None, OP.mult)
                nc.vector.tensor_scalar(
                    var[:], var[:], 1.0, EPS, OP.mult, OP.add)
                nc.vector.tensor_scalar(
                    arb[0:p, ecol:ecol + 1], arb[0:p, ecol:ecol + 1],
                    inv, None, OP.mult)
                nc.vector.tensor_tensor(var[:], var[:],
                                        arb[0:p, ecol:ecol + 1], OP.add)
                rec = tiny.tile([p, 1], F32, tag=f"rec_{name}")
                nc.vector.reciprocal(rec[:], var[:])
                r = tiny.tile([p, 1], F32, tag=f"r_{name}")
                nc.scalar.activation(r[:], rec[:], AF.Sqrt)
                mr = tiny.tile([p, 1], F32, tag=f"mr_{name}")
                nc.vector.tensor_tensor(mr[:], mu[:], r[:], OP.mult)
                nmr = tiny.tile([p, 1], F32, tag=f"nmr_{name}")
                nc.vector.tensor_scalar(nmr[:], mr[:], -1.0, None, OP.mult)
                grp[name] = (r, mr, nmr)

            # psi bias: C = bpsi - sum_o wpsi_o * q_o
            # q_o = mr_g + mr_x + 2 (o0) ; mr_g + 2 (o1: x uses relu-form)
            q0 = tiny.tile([128, 1], BF16, tag="q0")
            qt = tiny.tile([128, 1], F32, tag="qt")
            nc.vector.tensor_tensor(qt[:], grp["g0"][1][:], grp["x0"][1][:],
                                    OP.add)
            nc.vector.tensor_scalar(q0[:], qt[:], 1.0, 2.0, OP.mult, OP.add)
            q1 = tiny.tile([32, 1], BF16, tag="q1")
            nc.vector.tensor_scalar(q1[:], grp["g1"][1][:], 1.0, 2.0,
                                    OP.mult, OP.add)
            dot = ps.tile([1, 1], F32, tag="pA")
            nc.tensor.matmul(dot[:], wp0[:], q0[:], start=True, stop=False)
            nc.tensor.matmul(dot[:], wp1[:], q1[:], start=False, stop=True)
            chalf = tiny.tile([1, 1], F32, tag="chalf")
            nc.vector.tensor_scalar(chalf[:], dot[:], -0.5, cbh[:],
                                    OP.mult, OP.add)

            r_g0, mr_g0, nmr_g0 = grp["g0"]
            r_g1, mr_g1, nmr_g1 = grp["g1"]
            r_x0, mr_x0, nmr_x0 = grp["x0"]
            r_x1, mr_x1, nmr_x1 = grp["x1"]

            # ================= Phase 2 =================
            for sc in range(NSC):
                s0, s1_ = sc * 4 * CH, (sc + 1) * 4 * CH
                # g-side pointwise at FD=2048 from storage
                eg0 = stage.tile([128, 4 * CH], BF16, tag="eg0")
                eg1 = stage.tile([32, 4 * CH], BF16, tag="eg1")
                ag0 = stage.tile([128, 4 * CH], BF16, tag="ag0")
                ag1 = stage.tile([32, 4 * CH], BF16, tag="ag1")
                nc.scalar.activation(eg0[:], yg0[:, s0:s1_], AF.Exp,
                                     bias=nmr_g0[:], scale=r_g0[:])
                nc.scalar.activation(eg1[:], yg1[:, s0:s1_], AF.Exp,
                                     bias=nmr_g1[:], scale=r_g1[:])
                nc.vector.tensor_scalar(ag0[:], yg0[:, s0:s1_], r_g0[:],
                                        mr_g0[:], OP.mult, OP.max)
                nc.vector.tensor_scalar(ag1[:], yg1[:, s0:s1_], r_g1[:],
                                        mr_g1[:], OP.mult, OP.max)
                # t = min(e,1) in place (gpsimd)
                nc.gpsimd.tensor_scalar(eg0[:], eg0[:], 1.0, 1.0,
                                        OP.mult, OP.min)
                nc.gpsimd.tensor_scalar(eg1[:], eg1[:], 1.0, 1.0,
                                        OP.mult, OP.min)
                # s_g = t + a, in place into ag
                nc.vector.tensor_tensor(ag0[:], eg0[:], ag0[:], OP.add)
                nc.vector.tensor_tensor(ag1[:], eg1[:], ag1[:], OP.add)

                ex0 = stage.tile([128, 4 * CH], BF16, tag="ex0")
                ex1 = stage.tile([32, 4 * CH], BF16, tag="ex1")
                ax0 = stage.tile([128, 4 * CH], BF16, tag="ax0")
                ax1 = stage.tile([32, 4 * CH], BF16, tag="ax1")
                xc2_tiles = []
                for k in range(4):
                    j = sc * 4 + k
                    w0, w1 = j * CH, (j + 1) * CH
                    k0, k1 = k * CH, (k + 1) * CH
                    xc2 = xc2p_pool.tile([64, CH], BF16, tag="xc2")
                    nc.sync.dma_start(xc2[:], x_d[n, 256:320, w0:w1])
                    xc2_tiles.append(xc2)
                    px0 = ps.tile([128, CH], F32, tag="pC")
                    nc.tensor.matmul(px0[:], wx0[:, 0:128], xs0[:, w0:w1],
                                     start=True, stop=False)
                    nc.tensor.matmul(px0[:], wx1[:, 0:128], xs1[:, w0:w1], start=False, stop=False)
                    nc.tensor.matmul(px0[:], wx2[:, 0:128], xc2[:], start=False, stop=True)
                    px1 = ps.tile([32, CH], F32, tag="pD")
                    nc.tensor.matmul(px1[:], wx0[:, 128:160], xs0[:, w0:w1],
                                     start=True, stop=False)
                    nc.tensor.matmul(px1[:], wx1[:, 128:160], xs1[:, w0:w1], start=False, stop=False)
                    nc.tensor.matmul(px1[:], wx2[:, 128:160], xc2[:], start=False, stop=True)
                    nc.scalar.activation(ex0[:, k0:k1], px0[:], AF.Exp,
                                         bias=nmr_x0[:], scale=r_x0[:])
                    nc.scalar.activation(ex1[:, k0:k1], px1[:], AF.Exp,
                                         bias=nmr_x1[:], scale=r_x1[:])
                    nc.vector.tensor_scalar(ax0[:, k0:k1], px0[:], r_x0[:],
                                            mr_x0[:], OP.mult, OP.max)
                    # x-o1 uses relu-form on ACT (shift folded into psi bias)
                    nc.scalar.activation(ax1[:, k0:k1], px1[:], AF.Relu,
                                         bias=nmr_x1[:], scale=r_x1[:])
                # t_x = min(e_x, 1); s_x = t + a (in place)
                nc.vector.tensor_scalar(ex0[:], ex0[:], 1.0, 1.0,
                                        OP.mult, OP.min)
                nc.vector.tensor_scalar(ex1[:], ex1[:], 1.0, 1.0,
                                        OP.mult, OP.min)
                nc.vector.tensor_tensor(ax0[:], ex0[:], ax0[:], OP.add)
                nc.vector.tensor_tensor(ax1[:], ex1[:], ax1[:], OP.add)

                for k in range(4):
                    j = sc * 4 + k
                    w0, w1 = j * CH, (j + 1) * CH
                    k0, k1 = k * CH, (k + 1) * CH
                    pp = ps.tile([1, CH], F32, tag="pA")
                    nc.tensor.matmul(pp[:], wp0[:], ag0[:, k0:k1], start=True, stop=False)
                    nc.tensor.matmul(pp[:], wp1[:], ag1[:, k0:k1], start=False, stop=False)
                    nc.tensor.matmul(pp[:], wp0[:], ax0[:, k0:k1], start=False, stop=False)
                    nc.tensor.matmul(pp[:], wp1[:], ax1[:, k0:k1], start=False, stop=True)
                    pt = psip.tile([1, CH], BF16, tag="pt")
                    nc.scalar.activation(pt[:], pp[:], AF.Tanh,
                                         bias=chalf[:], scale=0.5)
                    # psi = 0.5*tanh + 0.5
                    nc.vector.tensor_scalar(pt[:], pt[:], 0.5, 0.5,
                                            OP.mult, OP.add)
                    pb = psip.tile([128, CH], BF16, tag="pb")
                    nc.gpsimd.partition_broadcast(pb[:], pt[:])
                    ob0 = outp.tile([128, CH], F32, tag="ob0")
                    ob1 = outp.tile([128, CH], F32, tag="ob1")
                    ob2 = outp.tile([64, CH], F32, tag="ob2")
                    nc.vector.tensor_tensor(ob0[:], xs0[:, w0:w1], pb[:],
                                            OP.mult)
                    nc.gpsimd.tensor_tensor(ob1[:], xs1[:, w0:w1], pb[:],
                                            OP.mult)
                    nc.vector.tensor_tensor(ob2[:], xc2_tiles[k][:],
                                            pb[0:64, :], OP.mult)
                    nc.sync.dma_start(out_d[n, 0:128, w0:w1], ob0[:])
                    nc.sync.dma_start(out_d[n, 128:256, w0:w1], ob1[:])
                    nc.sync.dma_start(out_d[n, 256:320, w0:w1], ob2[:])

    nc.compile()
    return nc


_CACHE = {}


def _get_nc(S, n_cores):
    key = (S, n_cores)
    if key not in _CACHE:
        _CACHE[key] = build_kernel(S, n_cores)
    return _CACHE[key]


def kernel(g, x, Wg, bg, Wx, bx, Wpsi, bpsi):
    n, c, d, h, w = g.shape
    assert (n, c) == (NB, C)
    n_cores = N_CORES
    assert d % n_cores == 0
    dsh = d // n_cores
    S = dsh * h * w
    nc = _get_nc(S, n_cores)

    wgt = np.ascontiguousarray(Wg.T).astype(BF)
    wxt = np.ascontiguousarray(Wx.T).astype(BF)
    wpt = np.ascontiguousarray(Wpsi.reshape(1, O).T).astype(BF)
    cb = np.array([[float(np.asarray(bpsi).reshape(-1)[0]) * 0.5]],
                  dtype=np.float32)

    g5 = g.reshape(n, c, d, h * w)
    x5 = x.reshape(n, c, d, h * w)
    in_maps = []
    for cid in range(n_cores):
        dl, dh_ = cid * dsh, (cid + 1) * dsh
        gs = np.ascontiguousarray(g5[:, :, dl:dh_]).reshape(n, c, S).astype(BF)
        xsn = np.ascontiguousarray(x5[:, :, dl:dh_]).reshape(n, c, S).astype(BF)
        in_maps.append({
            "g": gs, "x": xsn,
            "wgt": wgt, "wxt": wxt, "wpt": wpt, "cb": cb,
        })

    res = run_bass_kernel_spmd(nc, in_maps, core_ids=list(range(n_cores)))

    out = np.empty((n, c, d, h * w), dtype=np.float32)
    for cid in range(n_cores):
        dl, dh_ = cid * dsh, (cid + 1) * dsh
        out[:, :, dl:dh_] = res.results[cid]["out"].reshape(n, c, dsh, h * w)
    return out.reshape(n, c, d, h, w)


# revision 11
# speedup vs baseline: 2.0498x; 2.0498x over previous
"""AttentionGate (conv1x1 + InstanceNorm + ELU gate) on 8 trn2 NeuronCores.

Strategy:
  - Shard the D axis (32) across 8 cores (4 planes each); batch n=2 handled
    sequentially per core.
  - Host converts g/x and weights to bf16 (halves HBM traffic; all matmuls
    run bf16 with f32 PSUM accumulation).
  - InstanceNorm bias cancels, so bg/bx are mathematically irrelevant.
  - Per n: Phase 1 computes y_g = Wg.T@g (stored bf16 in SBUF) and
    y_x = Wx.T@x (stats only), accumulating per-chunk bn_stats. A tiny
    AllReduce (8 cores) merges [mean, E[y^2]] per (n, channel). Phase 2
    recomputes y_x from the SBUF-stored bf16 x, applies
    elu(z) = max(z,0) + min(exp(z),1) - 1 with per-channel constants folded
    into the psi bias, computes psi = sigmoid(Wpsi.T@(g1+x1)+bpsi) via
    tanh (same ACT table as Exp), broadcasts psi over partitions, and
    writes out = x * psi.
"""

import sys

if "/opt/trn_rl_repo/concourse" not in sys.path:
    sys.path.insert(0, "/opt/trn_rl_repo/concourse")

import contextlib

import numpy as np
import ml_dtypes

import concourse.bass as bass
import concourse.bacc as bacc
import concourse.mybir as mybir
import concourse.tile as tile
from concourse.bass_utils import run_bass_kernel_spmd

F32 = mybir.dt.float32
BF16 = mybir.dt.bfloat16
BF = ml_dtypes.bfloat16
AF = mybir.ActivationFunctionType
OP = mybir.AluOpType

N_CORES = 8
NB = 2          # batch
C = 320         # input channels
O = 160         # inter channels
EPS = 1e-5
CH = 512        # pixels per chunk


def build_kernel(S, n_cores=N_CORES):
    """Build the per-core bass kernel. S = per-core pixels per batch entry."""
    NCH = S // CH          # chunks per n
    NSC = NCH // 4         # superchunks (4 chunks) per n
    assert S % (4 * CH) == 0

    nc = bacc.Bacc("TRN2", target_bir_lowering=False, debug=False,
                   num_devices=n_cores)

    g_d = nc.dram_tensor("g", [NB, C, S], BF16, kind="ExternalInput")
    x_d = nc.dram_tensor("x", [NB, C, S], BF16, kind="ExternalInput")
    wg_d = nc.dram_tensor("wgt", [C, O], BF16, kind="ExternalInput")
    wx_d = nc.dram_tensor("wxt", [C, O], BF16, kind="ExternalInput")
    wp_d = nc.dram_tensor("wpt", [O, 1], BF16, kind="ExternalInput")
    cb_d = nc.dram_tensor("cb", [1, 1], F32, kind="ExternalInput")
    out_d = nc.dram_tensor("out", [NB, C, S], F32, kind="ExternalOutput")

    ar_in = [nc.dram_tensor(f"ar_in{n}", [128, 8], F32, kind="Internal")
             for n in range(NB)]
    ar_out = [nc.dram_tensor(f"ar_out{n}", [128, 8], F32, kind="Internal",
                             addr_space="Shared")
              for n in range(NB)]

    with tile.TileContext(nc) as tc, contextlib.ExitStack() as ctx:
        cpool = ctx.enter_context(tc.tile_pool(name="cpool", bufs=1))
        store = ctx.enter_context(tc.tile_pool(name="store", bufs=1))
        stats = ctx.enter_context(tc.tile_pool(name="stats", bufs=1))
        inp = ctx.enter_context(tc.tile_pool(name="inp", bufs=2))
        inp1 = ctx.enter_context(tc.tile_pool(name="inp1", bufs=1))
        xc2p_pool = ctx.enter_context(tc.tile_pool(name="xc2p", bufs=2))
        stage = ctx.enter_context(tc.tile_pool(name="stage", bufs=1))
        psip = ctx.enter_context(tc.tile_pool(name="psip", bufs=2))
        outp = ctx.enter_context(tc.tile_pool(name="outp", bufs=1))
        tiny = ctx.enter_context(tc.tile_pool(name="tiny", bufs=1))
        ps = ctx.enter_context(tc.tile_pool(name="ps", bufs=2, space="PSUM"))

        # ---- constants / weights ----
        wg0 = cpool.tile([128, O], BF16, tag="wg0")
        wg1 = cpool.tile([128, O], BF16, tag="wg1")
        wg2 = cpool.tile([64, O], BF16, tag="wg2")
        wx0 = cpool.tile([128, O], BF16, tag="wx0")
        wx1 = cpool.tile([128, O], BF16, tag="wx1")
        wx2 = cpool.tile([64, O], BF16, tag="wx2")
        nc.sync.dma_start(wg0[:], wg_d[0:128, :])
        nc.sync.dma_start(wg1[:], wg_d[128:256, :])
        nc.sync.dma_start(wg2[:], wg_d[256:320, :])
        nc.sync.dma_start(wx0[:], wx_d[0:128, :])
        nc.sync.dma_start(wx1[:], wx_d[128:256, :])
        nc.sync.dma_start(wx2[:], wx_d[256:320, :])
        wp0 = cpool.tile([128, 1], BF16, tag="wp0")
        wp1 = cpool.tile([32, 1], BF16, tag="wp1")
        nc.sync.dma_start(wp0[:], wp_d[0:128, :])
        nc.sync.dma_start(wp1[:], wp_d[128:160, :])
        cbh = cpool.tile([1, 1], F32, tag="cbh")
        nc.sync.dma_start(cbh[:], cb_d[:])

        # ---- persistent per-n storage (reused; deps serialize) ----
        xs0 = store.tile([128, S], BF16, tag="xs0")
        xs1 = store.tile([128, S], BF16, tag="xs1")
        yg0 = store.tile([128, S], BF16, tag="yg0")
        yg1 = store.tile([32, S], BF16, tag="yg1")

        # stats collect tiles
        sg0 = stats.tile([128, NCH * 6], F32, tag="sg0")
        sg1 = stats.tile([32, NCH * 6], F32, tag="sg1")
        sx0 = stats.tile([128, NCH * 6], F32, tag="sx0")
        sx1 = stats.tile([32, NCH * 6], F32, tag="sx1")

        for n in range(NB):
            # ================= Phase 1 =================
            for sc in range(NSC):
              scw0, scw1 = sc * 4 * CH, (sc + 1) * 4 * CH
              ga = inp.tile([128, 4 * CH], BF16, tag="ga")
              gb = inp.tile([128, 4 * CH], BF16, tag="gb")
              gc = inp1.tile([64, 4 * CH], BF16, tag="gc")
              nc.sync.dma_start(ga[:], g_d[n, 0:128, scw0:scw1])
              nc.sync.dma_start(gb[:], g_d[n, 128:256, scw0:scw1])
              nc.sync.dma_start(gc[:], g_d[n, 256:320, scw0:scw1])
              nc.sync.dma_start(xs0[:, scw0:scw1], x_d[n, 0:128, scw0:scw1])
              nc.sync.dma_start(xs1[:, scw0:scw1], x_d[n, 128:256, scw0:scw1])
              xc2 = xc2p_pool.tile([64, 4 * CH], BF16, tag="xc2")
              nc.sync.dma_start(xc2[:], x_d[n, 256:320, scw0:scw1])
              for k in range(4):
                j = sc * 4 + k
                w0, w1 = j * CH, (j + 1) * CH
                k0, k1 = k * CH, (k + 1) * CH

                pg0 = ps.tile([128, CH], F32, tag="pA")
                nc.tensor.matmul(pg0[:], wg0[:, 0:128], ga[:, k0:k1], start=True, stop=False)
                nc.tensor.matmul(pg0[:], wg1[:, 0:128], gb[:, k0:k1], start=False, stop=False)
                nc.tensor.matmul(pg0[:], wg2[:, 0:128], gc[:, k0:k1], start=False, stop=True)
                pg1 = ps.tile([32, CH], F32, tag="pB")
                nc.tensor.matmul(pg1[:], wg0[:, 128:160], ga[:, k0:k1], start=True, stop=False)
                nc.tensor.matmul(pg1[:], wg1[:, 128:160], gb[:, k0:k1], start=False, stop=False)
                nc.tensor.matmul(pg1[:], wg2[:, 128:160], gc[:, k0:k1], start=False, stop=True)
                px0 = ps.tile([128, CH], F32, tag="pC")
                nc.tensor.matmul(px0[:], wx0[:, 0:128], xs0[:, w0:w1], start=True, stop=False)
                nc.tensor.matmul(px0[:], wx1[:, 0:128], xs1[:, w0:w1], start=False, stop=False)
                nc.tensor.matmul(px0[:], wx2[:, 0:128], xc2[:, k0:k1], start=False, stop=True)
                px1 = ps.tile([32, CH], F32, tag="pD")
                nc.tensor.matmul(px1[:], wx0[:, 128:160], xs0[:, w0:w1], start=True, stop=False)
                nc.tensor.matmul(px1[:], wx1[:, 128:160], xs1[:, w0:w1], start=False, stop=False)
                nc.tensor.matmul(px1[:], wx2[:, 128:160], xc2[:, k0:k1], start=False, stop=True)

                nc.vector.bn_stats(sg0[:, j * 6:(j + 1) * 6], pg0[:])
                nc.vector.bn_stats(sg1[:, j * 6:(j + 1) * 6], pg1[:])
                nc.vector.bn_stats(sx0[:, j * 6:(j + 1) * 6], px0[:])
                nc.vector.bn_stats(sx1[:, j * 6:(j + 1) * 6], px1[:])

                nc.scalar.activation(yg0[:, w0:w1], pg0[:], AF.Copy)
                nc.scalar.activation(yg1[:, w0:w1], pg1[:], AF.Copy)

            # ================= stats + AllReduce =================
            mv_g0 = tiny.tile([128, 2], F32, tag="mv_g0")
            mv_g1 = tiny.tile([32, 2], F32, tag="mv_g1")
            mv_x0 = tiny.tile([128, 2], F32, tag="mv_x0")
            mv_x1 = tiny.tile([32, 2], F32, tag="mv_x1")
            nc.vector.bn_aggr(mv_g0[:], sg0[:].rearrange("p (n s) -> p n s", s=6))
            nc.vector.bn_aggr(mv_g1[:], sg1[:].rearrange("p (n s) -> p n s", s=6))
            nc.vector.bn_aggr(mv_x0[:], sx0[:].rearrange("p (n s) -> p n s", s=6))
            nc.vector.bn_aggr(mv_x1[:], sx1[:].rearrange("p (n s) -> p n s", s=6))

            arst = tiny.tile([128, 8], F32, tag="arst")
            nc.vector.memset(arst[:], 0.0)
            for (mv, mcol, ecol, p) in (
                (mv_g0, 0, 1, 128), (mv_x0, 2, 3, 128),
                (mv_g1, 4, 5, 32), (mv_x1, 6, 7, 32),
            ):
                nc.vector.tensor_copy(arst[0:p, mcol:mcol + 1], mv[:, 0:1])
                # E[y^2] = var + mean^2
                nc.vector.tensor_tensor(arst[0:p, ecol:ecol + 1], mv[:, 0:1],
                                        mv[:, 0:1], OP.mult)
                nc.vector.tensor_tensor(arst[0:p, ecol:ecol + 1],
                                        arst[0:p, ecol:ecol + 1],
                                        mv[:, 1:2], OP.add)

            nc.sync.dma_start(ar_in[n].ap(), arst[:])
            nc.gpsimd.collective_compute(
                "AllReduce", OP.add,
                replica_groups=[list(range(n_cores))],
                ins=[ar_in[n].ap().opt()],
                outs=[ar_out[n].ap().opt()],
            )
            arb = tiny.tile([128, 8], F32, tag="arb")
            nc.sync.dma_start(arb[:], ar_out[n].ap())

            # ---- per-group norm constants ----
            inv = 1.0 / n_cores
            grp = {}
            for (name, mcol, ecol, p) in (
                ("g0", 0, 1, 128), ("x0", 2, 3, 128),
                ("g1", 4, 5, 32), ("x1", 6, 7, 32),
            ):
                mu = tiny.tile([p, 1], F32, tag=f"mu_{name}")
                nc.vector.tensor_scalar(mu[:], arb[0:p, mcol:mcol + 1],
                                        inv, None, OP.mult)
                var = tiny.tile([p, 1], F32, tag=f"var_{name}")
                # var = E2/ncores - mu^2 + EPS
                nc.vector.tensor_tensor(var[:], mu[:], mu[:], OP.mult)
                nc.vector.tensor_scalar(var[:], var[:], -1.0, None, OP.mult)
                nc.vector.tensor_scalar(
                    var[:], var[:], 1.0, EPS, OP.mult, OP.add)
                nc.vector.tensor_scalar(
                    arb[0:p, ecol:ecol + 1], arb[0:p, ecol:ecol + 1],
                    inv, None, OP.mult)
                nc.vector.tensor_tensor(var[:], var[:],
                                        arb[0:p, ecol:ecol + 1], OP.add)
                rec = tiny.tile([p, 1], F32, tag=f"rec_{name}")
                nc.vector.reciprocal(rec[:], var[:])
                r = tiny.tile([p, 1], F32, tag=f"r_{name}")
                nc.scalar.activation(r[:], rec[:], AF.Sqrt)
                mr = tiny.tile([p, 1], F32, tag=f"mr_{name}")
                nc.vector.tensor_tensor(mr[:], mu[:], r[:], OP.mult)
                nmr = tiny.tile([p, 1], F32, tag=f"nmr_{name}")
                nc.vector.tensor_scalar(nmr[:], mr[:], -1.0, None, OP.mult)
                grp[name] = (r, mr, nmr)

            # psi bias: C = bpsi - sum_o wpsi_o * q_o
            # q_o = mr_g + mr_x + 2 (o0) ; mr_g + 2 (o1: x uses relu-form)
            q0 = tiny.tile([128, 1], BF16, tag="q0")
            qt = tiny.tile([128, 1], F32, tag="qt")
            nc.vector.tensor_tensor(qt[:], grp["g0"][1][:], grp["x0"][1][:],
                                    OP.add)
            nc.vector.tensor_scalar(q0[:], qt[:], 1.0, 2.0, OP.mult, OP.add)
            q1 = tiny.tile([32, 1], BF16, tag="q1")
            nc.vector.tensor_scalar(q1[:], grp["g1"][1][:], 1.0, 2.0,
                                    OP.mult, OP.add)
            dot = ps.tile([1, 1], F32, tag="pA")
            nc.tensor.matmul(dot[:], wp0[:], q0[:], start=True, stop=False)
            nc.tensor.matmul(dot[:], wp1[:], q1[:], start=False, stop=True)
            chalf = tiny.tile([1, 1], F32, tag="chalf")
            nc.vector.tensor_scalar(chalf[:], dot[:], -0.5, cbh[:],
                                    OP.mult, OP.add)

            r_g0, mr_g0, nmr_g0 = grp["g0"]
            r_g1, mr_g1, nmr_g1 = grp["g1"]
            r_x0, mr_x0, nmr_x0 = grp["x0"]
            r_x1, mr_x1, nmr_x1 = grp["x1"]

            # ================= Phase 2 =================
            for sc in range(NSC):
                s0, s1_ = sc * 4 * CH, (sc + 1) * 4 * CH
                # g-side pointwise at FD=2048 from storage
                eg0 = stage.tile([128, 4 * CH], BF16, tag="eg0")
                eg1 = stage.tile([32, 4 * CH], BF16, tag="eg1")
                ag0 = stage.tile([128, 4 * CH], BF16, tag="ag0")
                ag1 = stage.tile([32, 4 * CH], BF16, tag="ag1")
                nc.scalar.activation(eg0[:], yg0[:, s0:s1_], AF.Exp,
                                     bias=nmr_g0[:], scale=r_g0[:])
                nc.scalar.activation(eg1[:], yg1[:, s0:s1_], AF.Exp,
                                     bias=nmr_g1[:], scale=r_g1[:])
                nc.vector.tensor_scalar(ag0[:], yg0[:, s0:s1_], r_g0[:],
                                        mr_g0[:], OP.mult, OP.max)
                nc.vector.tensor_scalar(ag1[:], yg1[:, s0:s1_], r_g1[:],
                                        mr_g1[:], OP.mult, OP.max)
                # t = min(e,1) in place (gpsimd)
                nc.gpsimd.tensor_scalar(eg0[:], eg0[:], 1.0, 1.0,
                                        OP.mult, OP.min)
                nc.gpsimd.tensor_scalar(eg1[:], eg1[:], 1.0, 1.0,
                                        OP.mult, OP.min)
                # s_g = t + a, in place into ag
                nc.vector.tensor_tensor(ag0[:], eg0[:], ag0[:], OP.add)
                nc.vector.tensor_tensor(ag1[:], eg1[:], ag1[:], OP.add)

                ex0 = stage.tile([128, 4 * CH], BF16, tag="ex0")
                ex1 = stage.tile([32, 4 * CH], BF16, tag="ex1")
                ax0 = stage.tile([128, 4 * CH], BF16, tag="ax0")
                ax1 = stage.tile([32, 4 * CH], BF16, tag="ax1")
                xc2 = xc2p_pool.tile([64, 4 * CH], BF16, tag="xc2")
                nc.sync.dma_start(xc2[:], x_d[n, 256:320, s0:s1_])
                for k in range(4):
                    j = sc * 4 + k
                    w0, w1 = j * CH, (j + 1) * CH
                    k0, k1 = k * CH, (k + 1) * CH
                    px0 = ps.tile([128, CH], F32, tag="pC")
                    nc.tensor.matmul(px0[:], wx0[:, 0:128], xs0[:, w0:w1],
                                     start=True, stop=False)
                    nc.tensor.matmul(px0[:], wx1[:, 0:128], xs1[:, w0:w1], start=False, stop=False)
                    nc.tensor.matmul(px0[:], wx2[:, 0:128], xc2[:, k0:k1], start=False, stop=True)
                    px1 = ps.tile([32, CH], F32, tag="pD")
                    nc.tensor.matmul(px1[:], wx0[:, 128:160], xs0[:, w0:w1],
                                     start=True, stop=False)
                    nc.tensor.matmul(px1[:], wx1[:, 128:160], xs1[:, w0:w1], start=False, stop=False)
                    nc.tensor.matmul(px1[:], wx2[:, 128:160], xc2[:, k0:k1], start=False, stop=True)
                    nc.scalar.activation(ex0[:, k0:k1], px0[:], AF.Exp,
                                         bias=nmr_x0[:], scale=r_x0[:])
                    nc.scalar.activation(ex1[:, k0:k1], px1[:], AF.Exp,
                                         bias=nmr_x1[:], scale=r_x1[:])
                    nc.vector.tensor_scalar(ax0[:, k0:k1], px0[:], r_x0[:],
                                            mr_x0[:], OP.mult, OP.max)
                    # x-o1 uses relu-form on ACT (shift folded into psi bias)
                    nc.scalar.activation(ax1[:, k0:k1], px1[:], AF.Relu,
                                         bias=nmr_x1[:], scale=r_x1[:])
                # t_x = min(e_x, 1); s_x = t + a (in place)
                nc.vector.tensor_scalar(ex0[:], ex0[:], 1.0, 1.0,
                                        OP.mult, OP.min)
                nc.vector.tensor_scalar(ex1[:], ex1[:], 1.0, 1.0,
                                        OP.mult, OP.min)
                nc.vector.tensor_tensor(ax0[:], ex0[:], ax0[:], OP.add)
                nc.vector.tensor_tensor(ax1[:], ex1[:], ax1[:], OP.add)

                for k in range(4):
                    j = sc * 4 + k
                    w0, w1 = j * CH, (j + 1) * CH
                    k0, k1 = k * CH, (k + 1) * CH
                    pp = ps.tile([1, CH], F32, tag="pA")
                    nc.tensor.matmul(pp[:], wp0[:], ag0[:, k0:k1], start=True, stop=False)
                    nc.tensor.matmul(pp[:], wp1[:], ag1[:, k0:k1], start=False, stop=False)
                    nc.tensor.matmul(pp[:], wp0[:], ax0[:, k0:k1], start=False, stop=False)
                    nc.tensor.matmul(pp[:], wp1[:], ax1[:, k0:k1], start=False, stop=True)
                    pt = psip.tile([1, CH], BF16, tag="pt")
                    nc.scalar.activation(pt[:], pp[:], AF.Tanh,
                                         bias=chalf[:], scale=0.5)
                    # psi = 0.5*tanh + 0.5
                    nc.vector.tensor_scalar(pt[:], pt[:], 0.5, 0.5,
                                            OP.mult, OP.add)
                    pb = psip.tile([128, CH], BF16, tag="pb")
                    nc.gpsimd.partition_broadcast(pb[:], pt[:])
                    ob0 = outp.tile([128, CH], F32, tag="ob0")
                    ob1 = outp.tile([128, CH], F32, tag="ob1")
                    ob2 = outp.tile([64, CH], F32, tag="ob2")
                    nc.vector.tensor_tensor(ob0[:], xs0[:, w0:w1], pb[:],
                                            OP.mult)
                    nc.gpsimd.tensor_tensor(ob1[:], xs1[:, w0:w1], pb[:],
                                            OP.mult)
                    nc.vector.tensor_tensor(ob2[:], xc2[:, k0:k1],
                                            pb[0:64, :], OP.mult)
                    nc.scalar.dma_start(out_d[n, 0:128, w0:w1], ob0[:])
                    nc.scalar.dma_start(out_d[n, 128:256, w0:w1], ob1[:])
                    nc.sync.dma_start(out_d[n, 256:320, w0:w1], ob2[:])

    nc.compile()
    return nc


_CACHE = {}


def _get_nc(S, n_cores):
    key = (S, n_cores)
    if key not in _CACHE:
        _CACHE[key] = build_kernel(S, n_cores)
    return _CACHE[key]


def kernel(g, x, Wg, bg, Wx, bx, Wpsi, bpsi):
    n, c, d, h, w = g.shape
    assert (n, c) == (NB, C)
    n_cores = N_CORES
    assert d % n_cores == 0
    dsh = d // n_cores
    S = dsh * h * w
    nc = _get_nc(S, n_cores)

    wgt = np.ascontiguousarray(Wg.T).astype(BF)
    wxt = np.ascontiguousarray(Wx.T).astype(BF)
    wpt = np.ascontiguousarray(Wpsi.reshape(1, O).T).astype(BF)
    cb = np.array([[float(np.asarray(bpsi).reshape(-1)[0]) * 0.5]],
                  dtype=np.float32)

    g5 = g.reshape(n, c, d, h * w)
    x5 = x.reshape(n, c, d, h * w)
    in_maps = []
    for cid in range(n_cores):
        dl, dh_ = cid * dsh, (cid + 1) * dsh
        gs = g5[:, :, dl:dh_].astype(BF).reshape(n, c, S)
        xsn = x5[:, :, dl:dh_].astype(BF).reshape(n, c, S)
        in_maps.append({
            "g": gs, "x": xsn,
            "wgt": wgt, "wxt": wxt, "wpt": wpt, "cb": cb,
        })

    res = run_bass_kernel_spmd(nc, in_maps, core_ids=list(range(n_cores)))

    out = np.empty((n, c, d, h * w), dtype=np.float32)
    for cid in range(n_cores):
        dl, dh_ = cid * dsh, (cid + 1) * dsh
        out[:, :, dl:dh_] = res.results[cid]["out"].reshape(n, c, dsh, h * w)
    return out.reshape(n, c, d, h, w)
